# revision 1
# baseline (speedup 1.0000x reference)
"""CRF loss (forward-algorithm partition function) on 8 TRN2 cores.

Linear-domain meet-in-the-middle chain (see the original kernel for the
math), restructured so every matmul on a core uses the SAME stationary
weights: cores 0-3 run the forward half-chain, cores 4-7 the transposed
backward half-chain, and each core processes its 32-column batch group as
TWO independent 16-column chains interleaved on PE/DVE. Constant per-core
weights let walrus's ldw-opt drop the per-matmul LdWeights reload (~60ns)
from the dependency cycle, which dominated the old fwd+bwd-per-core layout;
the slot cost is now DVE-throughput-bound at two [128,16] multiplies
(~158ns/slot vs ~212ns, measured in paired A/B on hardware).

The program is identical on all cores (SPMD); the chain direction lives in
the DATA: winit = [W0 | W | state0] with W0 used only at slot 0 (fwd:
expT/expT/u0; bwd: I/expT^T/w0), and the emission slab supplies ones-rows
for the bwd pad/trailing slots. Slot counts are uniform at 513:
  fwd: 513 matmuls, emission rows 1..513
  bwd: identity pad + 512 matmuls, emission rows 1024..514 + trailing ones
Meet: z_b = u_final[:,b] . w_final[:,b] (host, f64).
"""

import os

import numpy as np

import concourse.bass as bass
from concourse import mybir
from concourse.bass_utils import run_bass_kernel_spmd

import concourse.bass_utils as _BU

if not getattr(_BU, "_crf_ldw_patched", False):
    _orig_run_command = _BU.run_command

    def _patched_run_command(argv, **kw):
        argv = [
            a.replace("--enable-ldw-opt=false", "--enable-ldw-opt=true").replace(
                "--enable-birsim=true", "--enable-birsim=false"
            )
            for a in argv
        ]
        return _orig_run_command(argv, **kw)

    _BU.run_command = _patched_run_command
    _BU._crf_ldw_patched = True


def _get_runner(nc, n_cores):
    if "runner" in _prog_cache:
        return _prog_cache["runner"]
    import jax
    from jax.sharding import Mesh, PartitionSpec
    from jax.experimental.shard_map import shard_map
    from concourse import bass2jax
    from concourse.bass2jax import _bass_exec_p, install_neuronx_cc_hook

    install_neuronx_cc_hook()
    partition_name = nc.partition_id_tensor.name if nc.partition_id_tensor else None
    in_names, out_names, out_avals, zero_outs = [], [], [], []
    for alloc in nc.m.functions[0].allocations:
        if not isinstance(alloc, mybir.MemoryLocationSet):
            continue
        name = alloc.memorylocations[0].name
        if alloc.kind == "ExternalInput":
            if name != partition_name:
                in_names.append(name)
        elif alloc.kind == "ExternalOutput":
            out_names.append(name)
            shape = tuple(alloc.tensor_shape)
            dtype = mybir.dt.np(alloc.dtype)
            out_avals.append(jax.core.ShapedArray(shape, dtype))
            zero_outs.append(np.zeros(shape, dtype))
    n_params = len(in_names)
    in_names_all = in_names + out_names
    if partition_name is not None:
        in_names_all.append(partition_name)

    def _body(*args):
        operands = list(args)
        if partition_name is not None:
            operands.append(bass2jax.partition_id_tensor())
        return tuple(
            _bass_exec_p.bind(
                *operands,
                out_avals=tuple(out_avals),
                in_names=tuple(in_names_all),
                out_names=tuple(out_names),
                lowering_input_output_aliases=(),
                sim_require_finite=True,
                sim_require_nnan=True,
                nc=nc,
            )
        )

    devices = jax.devices()[:n_cores]
    mesh = Mesh(np.asarray(devices), ("core",))
    nio = n_params + len(out_names)
    fn = jax.jit(
        shard_map(
            _body,
            mesh=mesh,
            in_specs=(PartitionSpec("core"),) * nio,
            out_specs=(PartitionSpec("core"),) * len(out_names),
            check_rep=False,
        ),
        keep_unused=True,
    )
    shard = jax.sharding.NamedSharding(mesh, PartitionSpec("core"))
    runner = (fn, in_names[:n_params], out_names, zero_outs, shard, jax)
    _prog_cache["runner"] = runner
    return runner


B, T, L = 128, 1024, 128
START, STOP = L - 2, L - 1
NCORES = 8
NGROUP = 4                  # batch groups; core g = fwd, core 4+g = bwd
GCOLS = B // NGROUP         # 32 batch columns per core
SLOTS = 513
HCOLS = GCOLS // 2          # 16 columns per chain; two chains per core
CSHIFT = 5.35
CHUNKS = [16, 49] + [64] * 7    # emission DMA chunks (slots per chunk)
COFF = np.cumsum([0] + CHUNKS).tolist()

LAST_EXEC_NS = None
LAST_RESULTS = None

MM_BF16 = os.environ.get("CRF_MM_DTYPE", "bf16") == "bf16"
try:
    import ml_dtypes  # noqa: F401
except ImportError:
    MM_BF16 = False

_prog_cache = {}


def _build_program(repeat=1):
    if ("nc", repeat) in _prog_cache:
        return _prog_cache[("nc", repeat)]
    nc = bass.Bass(disable_frame_to_traceback=True)
    f32 = mybir.dt.float32
    mdt = mybir.dt.bfloat16 if MM_BF16 else f32
    # winit = [W0 | W | state0]
    winit = nc.declare_dram_parameter(
        "winit", [L, 2 * L + GCOLS], mdt, isOutput=False
    )
    ej = nc.declare_dram_parameter("ej", [L, SLOTS * GCOLS], mdt, isOutput=False)
    # final states ship bf16; host converts
    uw = nc.declare_dram_parameter("uw", [L, GCOLS], mdt, isOutput=True)

    DPER = SLOTS                # muls per chain per repeat
    PPER = 2 * SLOTS            # matmuls (both chains) per repeat

    from contextlib import ExitStack

    with ExitStack() as ctx:
        w_t = ctx.enter_context(nc.sbuf_tensor("w_t", [L, 2 * L + GCOLS], mdt))
        echunks = [
            ctx.enter_context(nc.sbuf_tensor(f"ej{ci}", [L, n * GCOLS], mdt))
            for ci, n in enumerate(CHUNKS)
        ]
        uv = ctx.enter_context(nc.sbuf_tensor("uv", [L, 2 * GCOLS], mdt))
        psc = [
            [
                ctx.enter_context(nc.psum_tensor(f"ps{c}{i}", [L, HCOLS], f32))
                for i in range(2)
            ]
            for c in range(2)
        ]
        w_sem = ctx.enter_context(nc.semaphore("w_sem"))
        esems = [
            ctx.enter_context(nc.semaphore(f"ej{ci}_sem"))
            for ci in range(len(CHUNKS))
        ]
        pes = ctx.enter_context(nc.semaphore("pes"))
        dves = [
            ctx.enter_context(nc.semaphore("dve0")),
            ctx.enter_context(nc.semaphore("dve1")),
        ]
        out_sem = ctx.enter_context(nc.semaphore("out_sem"))
        block = ctx.enter_context(nc.Block())

        W0_ap = w_t[:, 0:L]
        W_ap = w_t[:, L : 2 * L]

        def s0_ap(c):
            s = 2 * L + c * HCOLS
            return w_t[:, s : s + HCOLS]

        def uv_ap(t, c):
            s = (t % 2) * GCOLS + c * HCOLS
            return uv[:, s : s + HCOLS]

        def ej_ap(t, c):
            ci = max(i for i in range(len(CHUNKS)) if COFF[i] <= t)
            off = t - COFF[ci]
            s = off * GCOLS + c * HCOLS
            return ci, echunks[ci][:, s : s + HCOLS]

        @block.sync
        def _(sync):
            sync.dma_start(out=w_t[:, :], in_=winit[:, :]).then_inc(w_sem, 16)
            for ci, n in enumerate(CHUNKS):
                s = COFF[ci] * GCOLS
                sync.dma_start(
                    out=echunks[ci][:, :], in_=ej[:, s : s + n * GCOLS]
                ).then_inc(esems[ci], 16)
            sync.wait_ge(dves[0], repeat * DPER)
            sync.wait_ge(dves[1], repeat * DPER)
            # final states live in uv parity (SLOTS-1)%2 == 0 -> cols 0:GCOLS
            sync.dma_start(out=uw[:, :], in_=uv[:, 0:GCOLS]).then_inc(out_sem, 16)
            sync.wait_ge(out_sem, 16)

        @block.tensor
        def _(tensor):
            for r in range(repeat):
                bd = r * DPER
                for t in range(SLOTS):
                    lhsT = W0_ap if t == 0 else W_ap
                    for c in range(2):
                        rhs = s0_ap(c) if t == 0 else uv_ap(t - 1, c)
                        mm = nc.tensor.matmul(
                            psc[c][t % 2][:, :], lhsT, rhs, start=True, stop=True
                        ).then_inc(pes, 1)
                        if t == 0 and r == 0:
                            if c == 0:
                                mm._wait_ge(w_sem, 16)
                        else:
                            mm._wait_ge(dves[c], bd + t)

        @block.vector
        def _(vector):
            for r in range(repeat):
                bp = r * PPER
                for t in range(SLOTS):
                    for c in range(2):
                        ci, eap = ej_ap(t, c)
                        if t == COFF[ci] and r == 0 and c == 0:
                            vector.wait_ge(esems[ci], 16)
                        nc.vector.tensor_mul(
                            uv_ap(t, c), psc[c][t % 2][:, :], eap
                        ).then_inc(dves[c], 1)._wait_ge(pes, bp + 2 * t + c + 1)

    _prog_cache[("nc", repeat)] = nc
    return nc


def kernel(pred, transitions, tags, seq_len):
    global LAST_EXEC_NS, LAST_RESULTS
    pred = np.asarray(pred, dtype=np.float32)
    transitions = np.asarray(transitions, dtype=np.float32)
    tags = np.asarray(tags).astype(np.int64)
    seq_len = np.asarray(seq_len).astype(np.int64)

    c2 = float(transitions[STOP, STOP])
    stoprow = np.zeros(L, np.float32)
    stoprow[STOP] = np.exp(-c2)

    if MM_BF16:
        import ml_dtypes

        edt = ml_dtypes.bfloat16
    else:
        edt = np.float32

    expT = np.exp(transitions, dtype=np.float32)
    ident = np.eye(L, dtype=np.float32)
    u0 = np.zeros((L, GCOLS), np.float32)
    u0[START, :] = 1.0
    w0 = np.zeros((L, GCOLS), np.float32)
    w0[STOP, :] = 1.0

    winit_f = np.concatenate([expT, expT, u0], axis=1).astype(edt)
    winit_b = np.concatenate(
        [ident, np.ascontiguousarray(expT.T), w0], axis=1
    ).astype(edt)

    def _group_slabs(g):
        bs = g * GCOLS
        ecore = np.empty((GCOLS, T + 1, L), np.float32)
        np.exp(pred[bs : bs + GCOLS] - CSHIFT, out=ecore[:, :T, :])
        ecore[:, :T, START] = 0.0
        ecore[:, :T, STOP] = 0.0
        ecore[:, T, :] = stoprow
        for j in range(GCOLS):
            n = seq_len[bs + j]
            if n < T:
                ecore[j, n:T, :] = stoprow
        # fwd: e_t = ecore row t, t = 0..512
        ef = np.empty((L, SLOTS, GCOLS), edt)
        ef[:] = ecore[:, :SLOTS, :].transpose(2, 1, 0)
        # bwd: e_0 = ones; e_t = ecore row 1024-t (t=1..511); e_512 = ones
        eb = np.empty((L, SLOTS, GCOLS), edt)
        eb[:, 0, :] = 1.0
        eb[:, 512, :] = 1.0
        idx = 1024 - np.arange(1, 512)
        eb[:, 1:512, :] = ecore[:, idx, :].transpose(2, 1, 0)
        return ef.reshape(L, SLOTS * GCOLS), eb.reshape(L, SLOTS * GCOLS)

    from concurrent.futures import ThreadPoolExecutor

    with ThreadPoolExecutor(NGROUP) as pool:
        slabs = list(pool.map(_group_slabs, range(NGROUP)))

    core_ids = list(range(NCORES))
    in_maps = []
    for c in core_ids:
        if c < NGROUP:
            in_maps.append({"ej": slabs[c][0], "winit": winit_f})
        else:
            in_maps.append({"ej": slabs[c - NGROUP][1], "winit": winit_b})

    global _last_in_maps
    _last_in_maps = in_maps
    nc = _build_program()
    try:
        fn, names, out_names, zero_outs, shard, jax = _get_runner(nc, NCORES)
        dev_in = [
            jax.device_put(
                np.concatenate(
                    [np.asarray(in_maps[c][nm]) for c in core_ids], axis=0
                ),
                shard,
            )
            for nm in names
        ]
        dev_zero = [
            jax.device_put(np.concatenate([z] * NCORES, axis=0), shard)
            for z in zero_outs
        ]
        outs = fn(*dev_in, *dev_zero)
        glob = {nm: np.asarray(o) for nm, o in zip(out_names, outs)}
        results = [
            {nm: glob[nm][c * L : (c + 1) * L] for nm in out_names}
            for c in core_ids
        ]

        class _Res:
            pass

        res = _Res()
        res.results = results
        res.exec_time_ns = None
    except Exception:
        res = run_bass_kernel_spmd(nc, in_maps, core_ids)
    LAST_EXEC_NS = res.exec_time_ns
    LAST_RESULTS = res

    zmid = np.concatenate(
        [
            np.einsum(
                "ib,ib->b",
                res.results[g]["uw"].astype(np.float64),
                res.results[NGROUP + g]["uw"].astype(np.float64),
            )
            for g in range(NGROUP)
        ]
    )
    scores = np.log(zmid) + CSHIFT * seq_len
    pred_paths = scores.sum()

    emit = np.take_along_axis(pred, tags[:, :, None], axis=2)[:, :, 0]
    mask = np.arange(T)[None, :] < seq_len[:, None]
    real = (emit * mask).sum(dtype=np.float64)

    padded_tags = np.concatenate(
        [np.full((B, 1), START, np.int64), tags, np.zeros((B, 1), np.int64)], axis=1
    )
    padded_tags[np.arange(B), seq_len + 1] = STOP
    tr = transitions[padded_tags[:, :-1], padded_tags[:, 1:]]
    tmask = np.arange(T + 1)[None, :] < (seq_len + 1)[:, None]
    real += (tr * tmask).sum(dtype=np.float64)

    return np.float32(pred_paths - real)



# revision 12
# speedup vs baseline: 23.2857x; 23.2857x over previous
"""CRF loss (forward-algorithm partition function) on 8 TRN2 cores.

Parallel-segment linear-domain chain. The serial forward recursion
  u_{t+1} = (M'^T u_t) . e_t,   M' = exp(transitions - CSHIFT)
is broken into K independent segments per batch element using the
Perron-Frobenius contraction of products of positive matrices: each
segment k >= 1 starts from an all-ones vector W steps before its
checkpoint c_k = k*S; after W warmup steps the state direction matches
the true forward state at c_k to ~1e-5 (validated on host), and the
unknown per-segment scale telescopes out on the host via
  rho_k = <g_k, y_{k-1}> / <g_k, g_k>
where g_k (state at c_k) and y_k (state at c_{k+1}) are captured on
device. All (b, k) segment tasks are independent columns, so each core
runs a few lockstep chains of ~36 slots instead of 513 serial slots.

Per slot each chain does one [128, n] matmul (stationary bf16 weights
M', ldw-opt keeps them loaded) followed by an elementwise multiply with
that slot's emission column block. The multiplies are spread across
engines per-chain: 'dve' = DVE reads PSUM directly (1 instr), 'act' =
Activation copies PSUM->SBUF bf16 then DVE does a 4x-mode all-SBUF
multiply, 'pool' = GPSIMD reads PSUM directly. Segment 0 starts exactly
from u0 via a synthetic slot-0 emission u0/(M'^T 1) and "keeper"
emissions that hold the state at u0 for the rest of the warmup region.

The final padded step (stoprow at row T) and the rho/logz combine run
on the host in f64; loss = sum_b log z_b + CSHIFT*n_b - real_path.
"""

import os

import numpy as np

import concourse.bass as bass
from concourse import mybir
from concourse.bass_utils import run_bass_kernel_spmd

import concourse.bass_utils as _BU

if not getattr(_BU, "_crf_ldw_patched", False):
    _orig_run_command = _BU.run_command

    def _patched_run_command(argv, **kw):
        argv = [
            a.replace("--enable-ldw-opt=false", "--enable-ldw-opt=true").replace(
                "--enable-birsim=true", "--enable-birsim=false"
            )
            for a in argv
        ]
        return _orig_run_command(argv, **kw)

    _BU.run_command = _patched_run_command
    _BU._crf_ldw_patched = True


def _get_runner(nc, n_cores):
    if "runner" in _prog_cache:
        return _prog_cache["runner"]
    import jax
    from jax.sharding import Mesh, PartitionSpec
    from jax.experimental.shard_map import shard_map
    from concourse import bass2jax
    from concourse.bass2jax import _bass_exec_p, install_neuronx_cc_hook

    install_neuronx_cc_hook()
    partition_name = nc.partition_id_tensor.name if nc.partition_id_tensor else None
    in_names, out_names, out_avals, zero_outs = [], [], [], []
    for alloc in nc.m.functions[0].allocations:
        if not isinstance(alloc, mybir.MemoryLocationSet):
            continue
        name = alloc.memorylocations[0].name
        if alloc.kind == "ExternalInput":
            if name != partition_name:
                in_names.append(name)
        elif alloc.kind == "ExternalOutput":
            out_names.append(name)
            shape = tuple(alloc.tensor_shape)
            dtype = mybir.dt.np(alloc.dtype)
            out_avals.append(jax.core.ShapedArray(shape, dtype))
            zero_outs.append(np.zeros(shape, dtype))
    n_params = len(in_names)
    in_names_all = in_names + out_names
    if partition_name is not None:
        in_names_all.append(partition_name)

    def _body(*args):
        operands = list(args)
        if partition_name is not None:
            operands.append(bass2jax.partition_id_tensor())
        return tuple(
            _bass_exec_p.bind(
                *operands,
                out_avals=tuple(out_avals),
                in_names=tuple(in_names_all),
                out_names=tuple(out_names),
                lowering_input_output_aliases=(),
                sim_require_finite=True,
                sim_require_nnan=True,
                nc=nc,
            )
        )

    devices = jax.devices()[:n_cores]
    mesh = Mesh(np.asarray(devices), ("core",))
    nio = n_params + len(out_names)
    fn = jax.jit(
        shard_map(
            _body,
            mesh=mesh,
            in_specs=(PartitionSpec("core"),) * nio,
            out_specs=(PartitionSpec("core"),) * len(out_names),
            check_rep=False,
        ),
        keep_unused=True,
    )
    shard = jax.sharding.NamedSharding(mesh, PartitionSpec("core"))
    runner = (fn, in_names[:n_params], out_names, zero_outs, shard, jax)
    _prog_cache["runner"] = runner
    return runner


B, T, L = 128, 1024, 128
START, STOP = L - 2, L - 1
NCORES = 8
CSHIFT = 5.35

K = 32                       # time segments
S = T // K                   # payload steps per segment
W = 4                        # warmup steps
SLOTS = W + S                # chain length
TPC = K * B // NCORES        # segment-task columns per core

# per-core chains: (mode, ncols); modes: dve | act | pool
_chain_env = os.environ.get("CRF_CHAINS", "dve:96,dve:96,act:160,act:160")
CHAINS = [(m, int(n)) for m, n in (c.split(":") for c in _chain_env.split(","))]
assert sum(n for _, n in CHAINS) == TPC, (CHAINS, TPC)
NC = len(CHAINS)
COFFS = np.cumsum([0] + [n for _, n in CHAINS]).tolist()
ONES_W = max(n for _, n in CHAINS)

# emission DMA chunks, in slots
CHUNK_SLOTS = [1, 1, 2, 4, 4, 8, 8, 8]
assert sum(CHUNK_SLOTS) == SLOTS
CH_OFF = np.cumsum([0] + CHUNK_SLOTS).tolist()

LAST_EXEC_NS = None
LAST_RESULTS = None

_prog_cache = {}


def _build_program(repeat=1):
    if ("nc", repeat) in _prog_cache:
        return _prog_cache[("nc", repeat)]
    nc = bass.Bass(disable_frame_to_traceback=True)
    f32 = mybir.dt.float32
    bf16 = mybir.dt.bfloat16
    # winit = [M' | ones]
    winit = nc.declare_dram_parameter(
        "winit", [L, L + ONES_W], bf16, isOutput=False
    )
    ej = nc.declare_dram_parameter("ej", [L, SLOTS * TPC], bf16, isOutput=False)
    uw = nc.declare_dram_parameter("uw", [L, 2 * TPC], bf16, isOutput=True)

    from contextlib import ExitStack

    with ExitStack() as ctx:
        w_t = ctx.enter_context(nc.sbuf_tensor("w_t", [L, L + ONES_W], bf16))
        echunks = [
            ctx.enter_context(nc.sbuf_tensor(f"ej{ci}", [L, n * TPC], bf16))
            for ci, n in enumerate(CHUNK_SLOTS)
        ]
        uv = [
            ctx.enter_context(nc.sbuf_tensor(f"uv{c}", [L, 2 * n], bf16))
            for c, (_, n) in enumerate(CHAINS)
        ]
        tb = [
            ctx.enter_context(nc.sbuf_tensor(f"tb{c}", [L, 2 * n], bf16))
            if CHAINS[c][0] in ("act", "pool")
            else None
            for c, (_, n) in enumerate(CHAINS)
        ]
        gf = ctx.enter_context(nc.sbuf_tensor("gf", [L, 2 * TPC], bf16))
        psc = [
            [
                ctx.enter_context(nc.psum_tensor(f"ps{c}_{i}", [L, n], f32))
                for i in range(2)
            ]
            for c, (_, n) in enumerate(CHAINS)
        ]
        w_sem = ctx.enter_context(nc.semaphore("w_sem"))
        esems = [
            ctx.enter_context(nc.semaphore(f"e{ci}_sem"))
            for ci in range(len(CHUNK_SLOTS))
        ]
        pes = ctx.enter_context(nc.semaphore("pes"))
        ssems = [
            ctx.enter_context(nc.semaphore(f"s{c}_sem")) for c in range(NC)
        ]
        asems = [
            ctx.enter_context(nc.semaphore(f"a{c}_sem"))
            if CHAINS[c][0] in ("act", "pool")
            else None
            for c in range(NC)
        ]
        out_sem = ctx.enter_context(nc.semaphore("out_sem"))
        block = ctx.enter_context(nc.Block())

        W_ap = w_t[:, 0:L]

        def ones_ap(c):
            return w_t[:, L : L + CHAINS[c][1]]

        def uv_ap(t, c):
            n = CHAINS[c][1]
            s = (t % 2) * n
            return uv[c][:, s : s + n]

        def tb_ap(t, c):
            n = CHAINS[c][1]
            s = (t % 2) * n
            return tb[c][:, s : s + n]

        def g_ap(c):
            return gf[:, COFFS[c] : COFFS[c] + CHAINS[c][1]]

        def f_ap(c):
            return gf[:, TPC + COFFS[c] : TPC + COFFS[c] + CHAINS[c][1]]

        def ej_ap(t, c):
            ci = max(i for i in range(len(CHUNK_SLOTS)) if CH_OFF[i] <= t)
            off = (t - CH_OFF[ci]) * TPC + COFFS[c]
            return ci, echunks[ci][:, off : off + CHAINS[c][1]]

        def mm_rhs(t, c):
            if t == 0:
                return ones_ap(c)
            if t == W:
                return g_ap(c)
            return uv_ap(t - 1, c)

        def mul_dst(t, c):
            if t == W - 1:
                return g_ap(c)
            if t == SLOTS - 1:
                return f_ap(c)
            return uv_ap(t, c)

        @block.sync
        def _(sync):
            sync.dma_start(out=w_t[:, :], in_=winit[:, :]).then_inc(w_sem, 16)
            for ci, n in enumerate(CHUNK_SLOTS):
                s = CH_OFF[ci] * TPC
                sync.dma_start(
                    out=echunks[ci][:, :], in_=ej[:, s : s + n * TPC]
                ).then_inc(esems[ci], 16)
            for c in range(NC):
                sync.wait_ge(ssems[c], repeat * SLOTS)
            sync.dma_start(out=uw[:, :], in_=gf[:, :]).then_inc(out_sem, 16)
            sync.wait_ge(out_sem, 16)

        @block.tensor
        def _(tensor):
            for r in range(repeat):
                for t in range(SLOTS):
                    for c in range(NC):
                        mm = nc.tensor.matmul(
                            psc[c][t % 2][:, :],
                            W_ap,
                            mm_rhs(t, c),
                            start=True,
                            stop=True,
                        ).then_inc(pes, 1)
                        if t == 0 and r == 0:
                            if c == 0:
                                mm._wait_ge(w_sem, 16)
                        else:
                            mm._wait_ge(ssems[c], r * SLOTS + t)

        @block.vector
        def _(vector):
            for r in range(repeat):
                for t in range(SLOTS):
                    if r == 0:
                        ci = max(
                            i for i in range(len(CHUNK_SLOTS)) if CH_OFF[i] <= t
                        )
                        if t == CH_OFF[ci]:
                            vector.wait_ge(esems[ci], 16)
                    base = r * SLOTS * NC + t * NC
                    # direct-DVE chains first (ready earliest), then act chains
                    for c in range(NC):
                        if CHAINS[c][0] != "dve":
                            continue
                        _, eap = ej_ap(t, c)
                        nc.vector.tensor_mul(
                            mul_dst(t, c), psc[c][t % 2][:, :], eap
                        ).then_inc(ssems[c], 1)._wait_ge(pes, base + c + 1)
                    for c in range(NC):
                        if CHAINS[c][0] != "act":
                            continue
                        _, eap = ej_ap(t, c)
                        nc.vector.tensor_mul(
                            mul_dst(t, c), tb_ap(t, c), eap
                        ).then_inc(ssems[c], 1)._wait_ge(asems[c], r * SLOTS + t + 1)

        @block.scalar
        def _(scalar):
            for r in range(repeat):
                for t in range(SLOTS):
                    base = r * SLOTS * NC + t * NC
                    for c in range(NC):
                        if CHAINS[c][0] not in ("act", "pool"):
                            continue
                        nc.scalar.activation(
                            tb_ap(t, c),
                            psc[c][t % 2][:, :],
                            mybir.ActivationFunctionType.Copy,
                        ).then_inc(asems[c], 1)._wait_ge(pes, base + c + 1)

        if any(m == "pool" for m, _ in CHAINS):

            @block.gpsimd
            def _(gpsimd):
                for r in range(repeat):
                    for t in range(SLOTS):
                        if r == 0:
                            ci = max(
                                i
                                for i in range(len(CHUNK_SLOTS))
                                if CH_OFF[i] <= t
                            )
                            if t == CH_OFF[ci]:
                                gpsimd.wait_ge(esems[ci], 16)
                        base = r * SLOTS * NC + t * NC
                        for c in range(NC):
                            if CHAINS[c][0] != "pool":
                                continue
                            _, eap = ej_ap(t, c)
                            nc.gpsimd.tensor_mul(
                                mul_dst(t, c), tb_ap(t, c), eap
                            ).then_inc(ssems[c], 1)._wait_ge(
                                asems[c], r * SLOTS + t + 1
                            )

    _prog_cache[("nc", repeat)] = nc
    return nc


def _host_prep(pred, transitions, seq_len):
    """Build winit slab and per-core emission slabs (bf16)."""
    import ml_dtypes

    bf16 = ml_dtypes.bfloat16
    c2 = float(transitions[STOP, STOP])
    Mp = np.exp(transitions.astype(np.float64) - CSHIFT).astype(np.float32)
    stoprow = np.zeros(L, np.float32)
    stoprow[STOP] = np.exp(CSHIFT - c2)

    winit = np.empty((L, L + ONES_W), np.float32)
    winit[:, :L] = Mp
    winit[:, L:] = 1.0
    winit = winit.astype(bf16)

    u0set = np.zeros(L, np.float32)
    u0set[START] = 1.0 / Mp[:, START].sum()
    keeper = np.zeros(L, np.float32)
    keeper[START] = 1.0 / Mp[START, START]

    # ecore rows 0..T (row T = stoprow), [B, T+1, L]
    def _build_core(core):
        # tasks tau = core*TPC + i ; k = tau // B ; b = tau % B
        tau = core * TPC + np.arange(TPC)
        kk, bb = tau // B, tau % B
        # row index consumed at slot t (for k>0 and k==0 payload region)
        tt = np.arange(SLOTS)
        rows = np.where(
            tt[None, :] < W,
            kk[:, None] * S - W + tt[None, :],
            kk[:, None] * S + tt[None, :] - W,
        )  # [TPC, SLOTS]; negative only where k==0, t<W (overwritten below)
        rows = np.clip(rows, 0, T)
        # gather emissions: [TPC, SLOTS, L]
        em = np.empty((TPC, SLOTS, L), np.float32)
        np.exp(pred[bb[:, None], rows, :], out=em, where=(rows < T)[:, :, None])
        em[:, :, START] = 0.0
        em[:, :, STOP] = 0.0
        # post-end and row-T slots -> stoprow
        n_b = seq_len[bb]  # [TPC]
        dead = rows >= n_b[:, None]  # [TPC, SLOTS]
        em[dead] = stoprow
        # segment-0 warmup: u0set then keepers
        k0 = kk == 0
        if k0.any():
            em[k0, 0, :] = u0set
            for t in range(1, W):
                em[k0, t, :] = keeper
        # [L, SLOTS, TPC] -> [L, SLOTS*TPC]
        return np.ascontiguousarray(em.transpose(2, 1, 0)).reshape(
            L, SLOTS * TPC
        ).astype(bf16)

    from concurrent.futures import ThreadPoolExecutor

    with ThreadPoolExecutor(NCORES) as pool:
        slabs = list(pool.map(_build_core, range(NCORES)))
    return winit, slabs, Mp


def kernel(pred, transitions, tags, seq_len):
    global LAST_EXEC_NS, LAST_RESULTS
    pred = np.asarray(pred, dtype=np.float32)
    transitions = np.asarray(transitions, dtype=np.float32)
    tags = np.asarray(tags).astype(np.int64)
    seq_len = np.asarray(seq_len).astype(np.int64)

    winit, slabs, Mp = _host_prep(pred, transitions, seq_len)

    core_ids = list(range(NCORES))
    in_maps = [{"ej": slabs[c], "winit": winit} for c in core_ids]
    global _last_in_maps
    _last_in_maps = in_maps

    nc = _build_program()
    try:
        fn, names, out_names, zero_outs, shard, jax = _get_runner(nc, NCORES)
        dev_in = [
            jax.device_put(
                np.concatenate(
                    [np.asarray(in_maps[c][nm]) for c in core_ids], axis=0
                ),
                shard,
            )
            for nm in names
        ]
        dev_zero = [
            jax.device_put(np.concatenate([z] * NCORES, axis=0), shard)
            for z in zero_outs
        ]
        outs = fn(*dev_in, *dev_zero)
        glob = {nm: np.asarray(o) for nm, o in zip(out_names, outs)}
        results = [
            {nm: glob[nm][c * L : (c + 1) * L] for nm in out_names}
            for c in core_ids
        ]

        class _Res:
            pass

        res = _Res()
        res.results = results
        res.exec_time_ns = None
    except Exception:
        res = run_bass_kernel_spmd(nc, in_maps, core_ids)
    LAST_EXEC_NS = res.exec_time_ns
    LAST_RESULTS = res

    # reassemble g,y: [K, B, L]
    g_all = np.empty((K, B, L), np.float64)
    y_all = np.empty((K, B, L), np.float64)
    for c in core_ids:
        uwc = res.results[c]["uw"].astype(np.float64)  # [L, 2*TPC]
        tau = c * TPC + np.arange(TPC)
        kk, bb = tau // B, tau % B
        g_all[kk, bb, :] = uwc[:, :TPC].T
        y_all[kk, bb, :] = uwc[:, TPC:].T

    logrho = np.zeros(B)
    for k in range(1, K):
        num = np.einsum("bl,bl->b", g_all[k], y_all[k - 1])
        den = np.einsum("bl,bl->b", g_all[k], g_all[k])
        logrho += np.log(num / den)
    c2 = float(transitions[STOP, STOP])
    vend_stop = (y_all[K - 1] @ Mp.astype(np.float64)[:, STOP]) * np.exp(
        CSHIFT - c2
    )
    logz = np.log(vend_stop) + logrho + CSHIFT * seq_len
    pred_paths = logz.sum()

    emit = np.take_along_axis(pred, tags[:, :, None], axis=2)[:, :, 0]
    mask = np.arange(T)[None, :] < seq_len[:, None]
    real = (emit * mask).sum(dtype=np.float64)
    padded_tags = np.concatenate(
        [np.full((B, 1), START, np.int64), tags, np.zeros((B, 1), np.int64)],
        axis=1,
    )
    padded_tags[np.arange(B), seq_len + 1] = STOP
    tr = transitions[padded_tags[:, :-1], padded_tags[:, 1:]]
    tmask = np.arange(T + 1)[None, :] < (seq_len + 1)[:, None]
    real += (tr * tmask).sum(dtype=np.float64)

    return np.float32(pred_paths - real)


# revision 15
# speedup vs baseline: 35.2225x; 1.5126x over previous
"""CRF loss (forward-algorithm partition function) on 8 TRN2 cores.

Parallel-segment linear-domain chain. The serial forward recursion
  u_{t+1} = (M'^T u_t) . e_t,   M' = exp(transitions - CSHIFT)
is broken into K independent segments per batch element using the
Perron-Frobenius contraction of products of positive matrices: each
segment k >= 1 starts from an all-ones vector W steps before its
checkpoint c_k = k*S; after W warmup steps the state direction matches
the true forward state at c_k to ~1e-5 (validated on host), and the
unknown per-segment scale telescopes out on the host via
  rho_k = <g_k, y_{k-1}> / <g_k, g_k>
where g_k (state at c_k) and y_k (state at c_{k+1}) are captured on
device. All (b, k) segment tasks are independent columns, so each core
runs a few lockstep chains of ~36 slots instead of 513 serial slots.

Per slot each chain does one [128, n] matmul (stationary bf16 weights
M', ldw-opt keeps them loaded) followed by an elementwise multiply with
that slot's emission column block. The multiplies are spread across
engines per-chain: 'dve' = DVE reads PSUM directly (1 instr), 'act' =
Activation copies PSUM->SBUF bf16 then DVE does a 4x-mode all-SBUF
multiply, 'pool' = GPSIMD reads PSUM directly. Segment 0 starts exactly
from u0 via a synthetic slot-0 emission u0/(M'^T 1) and "keeper"
emissions that hold the state at u0 for the rest of the warmup region.

The final padded step (stoprow at row T) and the rho/logz combine run
on the host in f64; loss = sum_b log z_b + CSHIFT*n_b - real_path.
"""

import os

import numpy as np

import concourse.bass as bass
from concourse import mybir
from concourse.bass_utils import run_bass_kernel_spmd

import concourse.bass_utils as _BU

if not getattr(_BU, "_crf_ldw_patched", False):
    _orig_run_command = _BU.run_command

    def _patched_run_command(argv, **kw):
        argv = [
            a.replace("--enable-ldw-opt=false", "--enable-ldw-opt=true").replace(
                "--enable-birsim=true", "--enable-birsim=false"
            )
            for a in argv
        ]
        return _orig_run_command(argv, **kw)

    _BU.run_command = _patched_run_command
    _BU._crf_ldw_patched = True


def _get_runner(nc, n_cores):
    if "runner" in _prog_cache:
        return _prog_cache["runner"]
    import jax
    from jax.sharding import Mesh, PartitionSpec
    from jax.experimental.shard_map import shard_map
    from concourse import bass2jax
    from concourse.bass2jax import _bass_exec_p, install_neuronx_cc_hook

    install_neuronx_cc_hook()
    partition_name = nc.partition_id_tensor.name if nc.partition_id_tensor else None
    in_names, out_names, out_avals, zero_outs = [], [], [], []
    for alloc in nc.m.functions[0].allocations:
        if not isinstance(alloc, mybir.MemoryLocationSet):
            continue
        name = alloc.memorylocations[0].name
        if alloc.kind == "ExternalInput":
            if name != partition_name:
                in_names.append(name)
        elif alloc.kind == "ExternalOutput":
            out_names.append(name)
            shape = tuple(alloc.tensor_shape)
            dtype = mybir.dt.np(alloc.dtype)
            out_avals.append(jax.core.ShapedArray(shape, dtype))
            zero_outs.append(np.zeros(shape, dtype))
    n_params = len(in_names)
    in_names_all = in_names + out_names
    if partition_name is not None:
        in_names_all.append(partition_name)

    def _body(*args):
        operands = list(args)
        if partition_name is not None:
            operands.append(bass2jax.partition_id_tensor())
        return tuple(
            _bass_exec_p.bind(
                *operands,
                out_avals=tuple(out_avals),
                in_names=tuple(in_names_all),
                out_names=tuple(out_names),
                lowering_input_output_aliases=(),
                sim_require_finite=True,
                sim_require_nnan=True,
                nc=nc,
            )
        )

    devices = jax.devices()[:n_cores]
    mesh = Mesh(np.asarray(devices), ("core",))
    nio = n_params + len(out_names)
    fn = jax.jit(
        shard_map(
            _body,
            mesh=mesh,
            in_specs=(PartitionSpec("core"),) * nio,
            out_specs=(PartitionSpec("core"),) * len(out_names),
            check_rep=False,
        ),
        keep_unused=True,
    )
    shard = jax.sharding.NamedSharding(mesh, PartitionSpec("core"))
    runner = (fn, in_names[:n_params], out_names, zero_outs, shard, jax)
    _prog_cache["runner"] = runner
    return runner


B, T, L = 128, 1024, 128
START, STOP = L - 2, L - 1
NCORES = 8
CSHIFT = 5.35

K = 32                       # time segments
S = T // K                   # payload steps per segment
W = 4                        # warmup steps
SLOTS = W + S                # chain length
TPC = K * B // NCORES        # segment-task columns per core

# per-core chains: (mode, ncols); modes: dve | act | pool
_chain_env = os.environ.get("CRF_CHAINS", "dve:96,dve:96,act:160,act:160")
CHAINS = [(m, int(n)) for m, n in (c.split(":") for c in _chain_env.split(","))]
assert sum(n for _, n in CHAINS) == TPC, (CHAINS, TPC)
NC = len(CHAINS)
COFFS = np.cumsum([0] + [n for _, n in CHAINS]).tolist()
ONES_W = max(n for _, n in CHAINS)

# emission DMA chunks, in slots
CHUNK_SLOTS = [1, 1, 2, 4, 4, 8, 8, 8]
assert sum(CHUNK_SLOTS) == SLOTS
CH_OFF = np.cumsum([0] + CHUNK_SLOTS).tolist()

LAST_EXEC_NS = None
LAST_RESULTS = None

_prog_cache = {}


def _build_program(repeat=1):
    if ("nc", repeat) in _prog_cache:
        return _prog_cache[("nc", repeat)]
    nc = bass.Bass(disable_frame_to_traceback=True)
    f32 = mybir.dt.float32
    bf16 = mybir.dt.bfloat16
    # winit = [M' | ones]
    winit = nc.declare_dram_parameter(
        "winit", [L, L + ONES_W], bf16, isOutput=False
    )
    ej = nc.declare_dram_parameter("ej", [L, SLOTS * TPC], bf16, isOutput=False)
    uw = nc.declare_dram_parameter("uw", [L, 2 * TPC], bf16, isOutput=True)

    from contextlib import ExitStack

    with ExitStack() as ctx:
        w_t = ctx.enter_context(nc.sbuf_tensor("w_t", [L, L + ONES_W], bf16))
        echunks = [
            ctx.enter_context(nc.sbuf_tensor(f"ej{ci}", [L, n * TPC], bf16))
            for ci, n in enumerate(CHUNK_SLOTS)
        ]
        uv = [
            ctx.enter_context(nc.sbuf_tensor(f"uv{c}", [L, 2 * n], bf16))
            for c, (_, n) in enumerate(CHAINS)
        ]
        tb = [
            ctx.enter_context(nc.sbuf_tensor(f"tb{c}", [L, 2 * n], bf16))
            if CHAINS[c][0] in ("act", "pool")
            else None
            for c, (_, n) in enumerate(CHAINS)
        ]
        gf = ctx.enter_context(nc.sbuf_tensor("gf", [L, 2 * TPC], bf16))
        psc = [
            [
                ctx.enter_context(nc.psum_tensor(f"ps{c}_{i}", [L, n], f32))
                for i in range(2)
            ]
            for c, (_, n) in enumerate(CHAINS)
        ]
        w_sem = ctx.enter_context(nc.semaphore("w_sem"))
        esems = [
            ctx.enter_context(nc.semaphore(f"e{ci}_sem"))
            for ci in range(len(CHUNK_SLOTS))
        ]
        pes = ctx.enter_context(nc.semaphore("pes"))
        ssems = [
            ctx.enter_context(nc.semaphore(f"s{c}_sem")) for c in range(NC)
        ]
        asems = [
            ctx.enter_context(nc.semaphore(f"a{c}_sem"))
            if CHAINS[c][0] in ("act", "pool")
            else None
            for c in range(NC)
        ]
        out_sem = ctx.enter_context(nc.semaphore("out_sem"))
        block = ctx.enter_context(nc.Block())

        W_ap = w_t[:, 0:L]

        def ones_ap(c):
            return w_t[:, L : L + CHAINS[c][1]]

        def uv_ap(t, c):
            n = CHAINS[c][1]
            s = (t % 2) * n
            return uv[c][:, s : s + n]

        def tb_ap(t, c):
            n = CHAINS[c][1]
            s = (t % 2) * n
            return tb[c][:, s : s + n]

        def g_ap(c):
            return gf[:, COFFS[c] : COFFS[c] + CHAINS[c][1]]

        def f_ap(c):
            return gf[:, TPC + COFFS[c] : TPC + COFFS[c] + CHAINS[c][1]]

        def ej_ap(t, c):
            ci = max(i for i in range(len(CHUNK_SLOTS)) if CH_OFF[i] <= t)
            off = (t - CH_OFF[ci]) * TPC + COFFS[c]
            return ci, echunks[ci][:, off : off + CHAINS[c][1]]

        def mm_rhs(t, c):
            if t == 0:
                return ones_ap(c)
            if t == W:
                return g_ap(c)
            return uv_ap(t - 1, c)

        def mul_dst(t, c):
            if t == W - 1:
                return g_ap(c)
            if t == SLOTS - 1:
                return f_ap(c)
            return uv_ap(t, c)

        @block.sync
        def _(sync):
            sync.dma_start(out=w_t[:, :], in_=winit[:, :]).then_inc(w_sem, 16)
            # emission DMAs re-issued every repeat so repeat-marginal time
            # includes the HBM traffic (chunk overwrite waits on readers)
            for r in range(repeat):
                for ci, n in enumerate(CHUNK_SLOTS):
                    if r > 0:
                        for c in range(NC):
                            sync.wait_ge(
                                ssems[c], (r - 1) * SLOTS + CH_OFF[ci + 1]
                            )
                    s = CH_OFF[ci] * TPC
                    sync.dma_start(
                        out=echunks[ci][:, :], in_=ej[:, s : s + n * TPC]
                    ).then_inc(esems[ci], 16)
            for c in range(NC):
                sync.wait_ge(ssems[c], repeat * SLOTS)
            sync.dma_start(out=uw[:, :], in_=gf[:, :]).then_inc(out_sem, 16)
            sync.wait_ge(out_sem, 16)

        @block.tensor
        def _(tensor):
            for r in range(repeat):
                for t in range(SLOTS):
                    for c in range(NC):
                        mm = nc.tensor.matmul(
                            psc[c][t % 2][:, :],
                            W_ap,
                            mm_rhs(t, c),
                            start=True,
                            stop=True,
                        ).then_inc(pes, 1)
                        if t == 0 and r == 0:
                            if c == 0:
                                mm._wait_ge(w_sem, 16)
                        else:
                            mm._wait_ge(ssems[c], r * SLOTS + t)

        @block.vector
        def _(vector):
            for r in range(repeat):
                for t in range(SLOTS):
                    ci = max(
                        i for i in range(len(CHUNK_SLOTS)) if CH_OFF[i] <= t
                    )
                    if t == CH_OFF[ci]:
                        vector.wait_ge(esems[ci], (r + 1) * 16)
                    base = r * SLOTS * NC + t * NC
                    # direct-DVE chains first (ready earliest), then act chains
                    for c in range(NC):
                        if CHAINS[c][0] != "dve":
                            continue
                        _, eap = ej_ap(t, c)
                        nc.vector.tensor_mul(
                            mul_dst(t, c), psc[c][t % 2][:, :], eap
                        ).then_inc(ssems[c], 1)._wait_ge(pes, base + c + 1)
                    for c in range(NC):
                        if CHAINS[c][0] != "act":
                            continue
                        _, eap = ej_ap(t, c)
                        nc.vector.tensor_mul(
                            mul_dst(t, c), tb_ap(t, c), eap
                        ).then_inc(ssems[c], 1)._wait_ge(asems[c], r * SLOTS + t + 1)

        @block.scalar
        def _(scalar):
            for r in range(repeat):
                for t in range(SLOTS):
                    base = r * SLOTS * NC + t * NC
                    for c in range(NC):
                        if CHAINS[c][0] not in ("act", "pool"):
                            continue
                        nc.scalar.activation(
                            tb_ap(t, c),
                            psc[c][t % 2][:, :],
                            mybir.ActivationFunctionType.Copy,
                        ).then_inc(asems[c], 1)._wait_ge(pes, base + c + 1)

        if any(m == "pool" for m, _ in CHAINS):

            @block.gpsimd
            def _(gpsimd):
                for r in range(repeat):
                    for t in range(SLOTS):
                        ci = max(
                            i for i in range(len(CHUNK_SLOTS)) if CH_OFF[i] <= t
                        )
                        if t == CH_OFF[ci]:
                            gpsimd.wait_ge(esems[ci], (r + 1) * 16)
                        base = r * SLOTS * NC + t * NC
                        for c in range(NC):
                            if CHAINS[c][0] != "pool":
                                continue
                            _, eap = ej_ap(t, c)
                            nc.gpsimd.tensor_mul(
                                mul_dst(t, c), tb_ap(t, c), eap
                            ).then_inc(ssems[c], 1)._wait_ge(
                                asems[c], r * SLOTS + t + 1
                            )

    _prog_cache[("nc", repeat)] = nc
    return nc


def _host_prep(pred, transitions, seq_len):
    """Build winit slab and per-core emission slabs (bf16)."""
    import ml_dtypes

    bf16 = ml_dtypes.bfloat16
    c2 = float(transitions[STOP, STOP])
    Mp = np.exp(transitions.astype(np.float64) - CSHIFT).astype(np.float32)
    stoprow = np.zeros(L, np.float32)
    stoprow[STOP] = np.exp(CSHIFT - c2)

    winit = np.empty((L, L + ONES_W), np.float32)
    winit[:, :L] = Mp
    winit[:, L:] = 1.0
    winit = winit.astype(bf16)

    u0set = np.zeros(L, np.float32)
    u0set[START] = 1.0 / Mp[:, START].sum()
    keeper = np.zeros(L, np.float32)
    keeper[START] = 1.0 / Mp[START, START]

    # ecore rows 0..T (row T = stoprow), [B, T+1, L]
    def _build_core(core):
        # tasks tau = core*TPC + i ; k = tau // B ; b = tau % B
        tau = core * TPC + np.arange(TPC)
        kk, bb = tau // B, tau % B
        # row index consumed at slot t (for k>0 and k==0 payload region)
        tt = np.arange(SLOTS)
        rows = np.where(
            tt[None, :] < W,
            kk[:, None] * S - W + tt[None, :],
            kk[:, None] * S + tt[None, :] - W,
        )  # [TPC, SLOTS]; negative only where k==0, t<W (overwritten below)
        rows = np.clip(rows, 0, T)
        # gather emissions: [TPC, SLOTS, L]
        em = np.empty((TPC, SLOTS, L), np.float32)
        np.exp(pred[bb[:, None], rows, :], out=em, where=(rows < T)[:, :, None])
        em[:, :, START] = 0.0
        em[:, :, STOP] = 0.0
        # post-end and row-T slots -> stoprow
        n_b = seq_len[bb]  # [TPC]
        dead = rows >= n_b[:, None]  # [TPC, SLOTS]
        em[dead] = stoprow
        # segment-0 warmup: u0set then keepers
        k0 = kk == 0
        if k0.any():
            em[k0, 0, :] = u0set
            for t in range(1, W):
                em[k0, t, :] = keeper
        # [L, SLOTS, TPC] -> [L, SLOTS*TPC]
        return np.ascontiguousarray(em.transpose(2, 1, 0)).reshape(
            L, SLOTS * TPC
        ).astype(bf16)

    from concurrent.futures import ThreadPoolExecutor

    with ThreadPoolExecutor(NCORES) as pool:
        slabs = list(pool.map(_build_core, range(NCORES)))
    return winit, slabs, Mp


def kernel(pred, transitions, tags, seq_len):
    global LAST_EXEC_NS, LAST_RESULTS
    pred = np.asarray(pred, dtype=np.float32)
    transitions = np.asarray(transitions, dtype=np.float32)
    tags = np.asarray(tags).astype(np.int64)
    seq_len = np.asarray(seq_len).astype(np.int64)

    winit, slabs, Mp = _host_prep(pred, transitions, seq_len)

    core_ids = list(range(NCORES))
    in_maps = [{"ej": slabs[c], "winit": winit} for c in core_ids]
    global _last_in_maps
    _last_in_maps = in_maps

    nc = _build_program()
    try:
        fn, names, out_names, zero_outs, shard, jax = _get_runner(nc, NCORES)
        dev_in = [
            jax.device_put(
                np.concatenate(
                    [np.asarray(in_maps[c][nm]) for c in core_ids], axis=0
                ),
                shard,
            )
            for nm in names
        ]
        dev_zero = [
            jax.device_put(np.concatenate([z] * NCORES, axis=0), shard)
            for z in zero_outs
        ]
        outs = fn(*dev_in, *dev_zero)
        glob = {nm: np.asarray(o) for nm, o in zip(out_names, outs)}
        results = [
            {nm: glob[nm][c * L : (c + 1) * L] for nm in out_names}
            for c in core_ids
        ]

        class _Res:
            pass

        res = _Res()
        res.results = results
        res.exec_time_ns = None
    except Exception:
        res = run_bass_kernel_spmd(nc, in_maps, core_ids)
    LAST_EXEC_NS = res.exec_time_ns
    LAST_RESULTS = res

    # reassemble g,y: [K, B, L]
    g_all = np.empty((K, B, L), np.float64)
    y_all = np.empty((K, B, L), np.float64)
    for c in core_ids:
        uwc = res.results[c]["uw"].astype(np.float64)  # [L, 2*TPC]
        tau = c * TPC + np.arange(TPC)
        kk, bb = tau // B, tau % B
        g_all[kk, bb, :] = uwc[:, :TPC].T
        y_all[kk, bb, :] = uwc[:, TPC:].T

    logrho = np.zeros(B)
    for k in range(1, K):
        num = np.einsum("bl,bl->b", g_all[k], y_all[k - 1])
        den = np.einsum("bl,bl->b", g_all[k], g_all[k])
        logrho += np.log(num / den)
    c2 = float(transitions[STOP, STOP])
    vend_stop = (y_all[K - 1] @ Mp.astype(np.float64)[:, STOP]) * np.exp(
        CSHIFT - c2
    )
    logz = np.log(vend_stop) + logrho + CSHIFT * seq_len
    pred_paths = logz.sum()

    emit = np.take_along_axis(pred, tags[:, :, None], axis=2)[:, :, 0]
    mask = np.arange(T)[None, :] < seq_len[:, None]
    real = (emit * mask).sum(dtype=np.float64)
    padded_tags = np.concatenate(
        [np.full((B, 1), START, np.int64), tags, np.zeros((B, 1), np.int64)],
        axis=1,
    )
    padded_tags[np.arange(B), seq_len + 1] = STOP
    tr = transitions[padded_tags[:, :-1], padded_tags[:, 1:]]
    tmask = np.arange(T + 1)[None, :] < (seq_len + 1)[:, None]
    real += (tr * tmask).sum(dtype=np.float64)

    return np.float32(pred_paths - real)
